# revision 25
# baseline (speedup 1.0000x reference)
"""DEDICOM decoder forward on 8 Trainium2 NeuronCores.

Math per relation k (k=0..7):
    M_k = diag(d_k) @ G @ diag(d_k)                  (64x64, host-precomputed)
    out[k, n] = sigmoid( (row_n @ M_k) . col_n )

Device algorithm (data-parallel over N across 8 cores; per core 62500
samples padded to 123*512=62976; block B = 512 samples, half h=s//256):

  Stage 1 (PE, row-tiled): per (block, half h, pair p in 0..3):
      y[(kappa,j), s] = sum_i Mpair_p[i, (kappa,j)] * rowT[i, s]
    matmul(lhsT=mq4[64h:64h+64, p, :] [64,128], rhs=rowc[64h:64h+64, b, :]
    [64,256]) -> PSUM f32 [128, 256]; pairs packed 2-per-bank
    ([128, 2, 256] f32 = one 2KB bank); h=0/1 run concurrently on the
    two 64-row PE tiles.

  Col-multiply U = Y * colT (colT kappa-duplicated on both partition
  halves, host-prepped), split across engines per block:
      DVE  : h0 banks, f32 PSUM-direct tensor_tensor -> SBUF bf16
      ACT  : h1 banks, bridge PSUM f32 -> SBUF bf16
      Pool : h1 pairs 0-1 bf16 mult (scalar_tensor_tensor)
      DVE  : h1 pairs 2-3 bf16 mult (scalar_tensor_tensor, 4x mode)

  Stage 2 (PE): reduce over j via constant selection matrix, ut chunk
  as stationary weights (FWL):
      matmul(lhsT=ut[:, p, 128c:128c+128] [128,128], rhs=sel [128,2])
    -> rec PSUM [128, 2] slices of a [128, 64, 8] bank (k = 2*pair+kappa).

  ACT: sigmoid per chunk [128, 64, 8] -> SBUF f32; DMA out.
"""

import sys

sys.path.insert(0, "/opt/trn_rl_repo")

import numpy as np
import ml_dtypes

import concourse.bass as bass
import concourse.bacc as bacc
import concourse.mybir as mybir
from concourse import tile
from concourse.bass_utils import run_bass_kernel_spmd

N, D, R = 500000, 64, 8
NCORES = 8
SHARD = N // NCORES            # 62500
BLK = 512                      # samples per block (256 per PE half)
NBLK = (SHARD + BLK - 1) // BLK  # 123
SHARD_PAD = NBLK * BLK         # 62976
CHUNK = 16                     # blocks per DMA chunk / rec bank
NCHUNK = (NBLK + CHUNK - 1) // CHUNK
BF16 = mybir.dt.bfloat16
F32 = mybir.dt.float32

_CACHE: dict = {}


def _build_program():
    if "nc" in _CACHE:
        return _CACHE["nc"]

    nc = bacc.Bacc(
        "TRN2", target_bir_lowering=False, debug=False, num_devices=NCORES
    )

    rowc_d = nc.dram_tensor("rowc", [128, NBLK, 256], BF16, kind="ExternalInput")
    colc_d = nc.dram_tensor("colc", [128, NBLK, 2, 256], BF16, kind="ExternalInput")
    mq_d = nc.dram_tensor("mq4", [128, 4 * 128], BF16, kind="ExternalInput")
    sel_d = nc.dram_tensor("sel", [128, 2], BF16, kind="ExternalInput")
    out_d = nc.dram_tensor("out", [128, NBLK, 4, R], F32, kind="ExternalOutput")

    MULT = mybir.AluOpType.mult

    with tile.TileContext(nc) as tc:
        with (
            tc.tile_pool(name="const", bufs=1) as cpool,
            tc.tile_pool(name="iorow", bufs=3) as rowpool,
            tc.tile_pool(name="iocol", bufs=3) as colpool,
            tc.tile_pool(name="ut0", bufs=4) as ut0pool,
            tc.tile_pool(name="ut1", bufs=4) as ut1pool,
            tc.tile_pool(name="ybf", bufs=3) as ybfpool,
            tc.tile_pool(name="sig", bufs=2) as sigpool,
            tc.tile_pool(name="py0", bufs=2, space="PSUM") as py0,
            tc.tile_pool(name="py1", bufs=1, space="PSUM") as py1,
            tc.tile_pool(name="psum_r", bufs=2, space="PSUM") as pr_pool,
        ):
            # mq4[64h+i, p, 64kap+j] = M_{2p+kap}[i, j]  (same for h=0/1)
            mq4 = cpool.tile([128, 4, 128], BF16, tag="mq4")
            sel = cpool.tile([128, 2], BF16, tag="sel")
            nc.sync.dma_start(mq4[:].rearrange("d p j -> d (p j)"), mq_d.ap())
            nc.sync.dma_start(sel[:], sel_d.ap())

            ypools = (py0, py1)

            # Software-pipelined emission across the flat block list:
            #   iter b: stage1(b) [PE] -> direct mults(b) [DVE] ->
            #           bridges(b) [ACT] -> bf16 mults(b-1) [Pool/DVE] ->
            #           stage2(b-2) [PE]
            # so no engine queue waits on a same-block producer.
            mult_q = []   # (ut1, ybf, col1) awaiting bf16 mults
            s2_q = []     # (ci, slot_base, ut0, ut1) awaiting stage 2
            chunks = {}   # ci -> (rec, nb, b0)

            def emit_bf16(bidx, ut1, ybf, col1):
                # Pool: h1 pairs 0-1; pairs 2-3 Pool on 2-of-5 blocks,
                # else DVE (balances DVE ~1.69us vs Pool ~1.62us per block)
                nc.gpsimd.tensor_tensor(
                    out=ut1[:, 0:2, :],
                    in0=ybf[:, 0:2, :],
                    in1=col1.broadcast_to([128, 2, 256]),
                    op=MULT,
                )
                eng = nc.gpsimd if (bidx % 5) < 2 else nc.vector
                eng.tensor_tensor(
                    out=ut1[:, 2:4, :],
                    in0=ybf[:, 2:4, :],
                    in1=col1.broadcast_to([128, 2, 256]),
                    op=MULT,
                )

            def emit_s2(ci, slot_base, ut0, ut1):
                rec = chunks[ci][0]
                for h, ut in ((0, ut0), (1, ut1)):
                    for c in range(2):
                        slot = slot_base + 2 * h + c
                        for p in range(4):
                            nc.tensor.matmul(
                                rec[:, slot, 2 * p : 2 * p + 2],
                                ut[:, p, 128 * c : 128 * c + 128],
                                sel[:],
                            )

            def emit_close(ci):
                rec, nb, b0 = chunks.pop(ci)
                sig = sigpool.tile([128, 4 * CHUNK, R], F32, tag="sig")
                nc.scalar.activation(
                    sig[:, 0 : 4 * nb, :],
                    rec[:, 0 : 4 * nb, :],
                    mybir.ActivationFunctionType.Sigmoid,
                )
                nc.sync.dma_start(
                    out_d.ap()[:, b0 : b0 + nb, :, :],
                    sig[:, 0 : 4 * nb, :].rearrange(
                        "n (b hc) k -> n b hc k", hc=4
                    ),
                )

            rowt = colt = None
            for gb in range(NBLK):
                ci, b = divmod(gb, CHUNK)
                if b == 0:
                    b0 = ci * CHUNK
                    nb = min(CHUNK, NBLK - b0)
                    rowt = rowpool.tile([128, CHUNK, 256], BF16, tag="rowt")
                    colt = colpool.tile([128, CHUNK, 2, 256], BF16, tag="colt")
                    nc.sync.dma_start(
                        rowt[:, 0:nb, :], rowc_d.ap()[:, b0 : b0 + nb, :]
                    )
                    nc.sync.dma_start(
                        colt[:, 0:nb, :, :],
                        colc_d.ap()[:, b0 : b0 + nb, :, :],
                    )
                    rec = pr_pool.tile(
                        [128, 4 * CHUNK, R], F32, tag="rec", name="rec"
                    )
                    chunks[ci] = (rec, nb, b0)

                # ---- stage 1: 8 matmuls, h halves run concurrently
                ys = []
                for h in range(2):
                    yduo = []
                    for duo in range(2):
                        y = ypools[h].tile(
                            [128, 2, 256], F32, tag=f"y{h}{duo}"
                        )
                        yduo.append(y)
                    ys.append(yduo)
                for duo in range(2):
                    for q in range(2):
                        p = 2 * duo + q
                        for h in range(2):
                            nc.tensor.matmul(
                                ys[h][duo][:, q, :],
                                mq4[64 * h : 64 * h + 64, p, :],
                                rowt[64 * h : 64 * h + 64, b, :],
                            )

                # ---- col multiply: U = Y * colT  (bf16 out)
                ut0 = ut0pool.tile([128, 4, 256], BF16, tag="ut0")
                ut1 = ut1pool.tile([128, 4, 256], BF16, tag="ut1")
                col0 = colt[:, b, 0, :].unsqueeze(1)
                col1 = colt[:, b, 1, :].unsqueeze(1)

                # DVE: h0 both duos, PSUM f32 direct
                for duo in range(2):
                    nc.vector.tensor_tensor(
                        out=ut0[:, 2 * duo : 2 * duo + 2, :],
                        in0=ys[0][duo][:],
                        in1=col0.broadcast_to([128, 2, 256]),
                        op=MULT,
                    )
                # ACT: bridge h1 banks PSUM f32 -> SBUF bf16
                ybf = ybfpool.tile([128, 4, 256], BF16, tag="ybf")
                for duo in range(2):
                    nc.scalar.copy(
                        ybf[:, 2 * duo : 2 * duo + 2, :], ys[1][duo][:]
                    )

                mult_q.append((gb, ut1, ybf, col1))
                s2_q.append((ci, 4 * b, ut0, ut1))

                # deferred work from earlier blocks
                if len(mult_q) > 1:
                    emit_bf16(*mult_q.pop(0))
                if len(s2_q) > 2:
                    e_ci, sb, u0, u1 = s2_q.pop(0)
                    emit_s2(e_ci, sb, u0, u1)
                    if not any(q[0] == e_ci for q in s2_q):
                        emit_close(e_ci)

            # drain
            while mult_q:
                emit_bf16(*mult_q.pop(0))
            while s2_q:
                e_ci, sb, u0, u1 = s2_q.pop(0)
                emit_s2(e_ci, sb, u0, u1)
                if not any(q[0] == e_ci for q in s2_q):
                    emit_close(e_ci)

    nc.compile()
    _CACHE["nc"] = nc
    return nc


def _prep_inputs(inputs_row, inputs_col, global_interaction, local_variation):
    d = np.asarray(local_variation, np.float32)
    g = np.asarray(global_interaction, np.float32)
    # mk[k, i, j] = d[k,i]*G[i,j]*d[k,j]
    mk = np.einsum("ki,ij,kj->kij", d, g, d)            # [8, 64, 64]
    # mq4[64h+i, p, 64kap+j] = mk[2p+kap, i, j]
    m4 = mk.reshape(4, 2, D, D).transpose(2, 0, 1, 3).reshape(D, 4, 2 * D)
    mq4 = np.concatenate([m4, m4], axis=0).reshape(128, 4 * 128)
    mq4 = np.ascontiguousarray(mq4).astype(ml_dtypes.bfloat16)

    sel = np.zeros((128, 2), np.float32)
    sel[0:64, 0] = 1.0
    sel[64:128, 1] = 1.0
    sel = sel.astype(ml_dtypes.bfloat16)

    row_f = np.asarray(inputs_row, np.float32)
    col_f = np.asarray(inputs_col, np.float32)
    pad = SHARD_PAD - SHARD
    in_maps = []
    for cidx in range(NCORES):
        sl = slice(cidx * SHARD, (cidx + 1) * SHARD)
        rr = np.concatenate(
            [row_f[sl], np.zeros((pad, D), np.float32)]
        ).astype(ml_dtypes.bfloat16)
        cc = np.concatenate(
            [col_f[sl], np.zeros((pad, D), np.float32)]
        ).astype(ml_dtypes.bfloat16)
        # rowc[64h+i, B, s] = row[B*512 + 256h + s, i]
        r4 = rr.reshape(NBLK, 2, 256, D).transpose(1, 3, 0, 2)  # [h, i, B, s]
        rowc = np.ascontiguousarray(r4.reshape(128, NBLK, 256))
        # colc[64kap+j, B, h, s] = col[B*512 + 256h + s, j]
        c4 = cc.reshape(NBLK, 2, 256, D).transpose(3, 0, 1, 2)  # [j, B, h, s]
        colc = np.ascontiguousarray(
            np.concatenate([c4, c4], axis=0)
        )  # [128, NBLK, 2, 256]
        in_maps.append(
            {"rowc": rowc, "colc": colc, "mq4": mq4, "sel": sel}
        )
    return in_maps


def kernel(inputs_row, inputs_col, global_interaction, local_variation):
    nc = _build_program()
    in_maps = _prep_inputs(
        inputs_row, inputs_col, global_interaction, local_variation
    )
    res = run_bass_kernel_spmd(nc, in_maps, list(range(NCORES)))
    outs = []
    for c in range(NCORES):
        o = res.results[c]["out"]                   # [128, NBLK, 4, 8]
        # sample = B*512 + hc*128 + p'
        o = o.transpose(1, 2, 0, 3).reshape(SHARD_PAD, R)
        outs.append(o[:SHARD])
    full = np.concatenate(outs, axis=0)             # [N, 8]
    return np.ascontiguousarray(full.T)             # [8, N]


if __name__ == "__main__":
    rng = np.random.default_rng(0)
    inputs = {
        "inputs_row": rng.standard_normal((N, D), dtype=np.float32),
        "inputs_col": rng.standard_normal((N, D), dtype=np.float32),
        "global_interaction": rng.uniform(-0.2, 0.2, (D, D)).astype(np.float32),
        "local_variation": rng.uniform(-0.3, 0.3, (R, D)).astype(np.float32),
    }
    out = kernel(**inputs)
    print("out", out.shape, out.dtype, out[:, :3])
